# revision 10
# baseline (speedup 1.0000x reference)
"""Trainium2 Bass kernel for nn_BlockTrainerBlend (8-core data parallel).

Math (per batch row):
  split x0/x1/x2 into C=20 chunks of S=80; per (modality m, chunk c):
  proj = x_chunk @ W[m,c]^T + b[m,c]  -> [R*S=400]
  m = proj0*proj1*proj2; z = sum over r -> [80]
  z' = signed-sqrt(z); z_norm = z'/max(||z'||, eps)
  chunk_logits[c] = z_norm[c] @ Wo_c^T + b_out; chunks_out = softmax
  final = softmax(z_flat @ W_out^T + b_out)

Sharding: batch (2048) split 8 ways -> 256 rows/core, two 128-row tiles.
Weights replicated. All matmul operands pre-transposed/cast to fp16 on host,
with a ones-row appended so biases ride inside the matmuls (K=81).
"""
import numpy as np

import concourse.bacc as bacc
import concourse.bass as bass
import concourse.tile as tile
from concourse import mybir
from concourse.bass_utils import run_bass_kernel_spmd
from concourse.masks import make_identity

B, MM, C, S, R, O = 2048, 1600, 20, 80, 5, 27
NCORES = 8
BL = B // NCORES          # 256 rows per core
NT = BL // 128            # 2 batch-tiles per core

F32 = mybir.dt.float32
F16 = mybir.dt.float16
AF = mybir.ActivationFunctionType
ALU = mybir.AluOpType

_prog = None  # cached compiled Bass program


def _emit(nc, tc, ctx):
    xT = nc.dram_tensor("xT", [81, 3, C, BL], F16, kind="ExternalInput").ap()
    Wb = nc.dram_tensor("Wb", [81, 3, C, R * S], F16, kind="ExternalInput").ap()
    WoT = nc.dram_tensor("WoT", [81, C, O], F16, kind="ExternalInput").ap()
    fin = nc.dram_tensor("fin", [BL, O], F32, kind="ExternalOutput").ap()
    chk = nc.dram_tensor("chk", [BL, C * O], F32, kind="ExternalOutput").ap()

    consts = ctx.enter_context(tc.tile_pool(name="consts", bufs=1))
    sb = ctx.enter_context(tc.tile_pool(name="sb", bufs=2))
    pp = ctx.enter_context(tc.tile_pool(name="pp", bufs=4, space="PSUM"))
    pt = ctx.enter_context(tc.tile_pool(name="pt", bufs=1, space="PSUM"))
    ph = ctx.enter_context(tc.tile_pool(name="ph", bufs=2, space="PSUM"))
    pf = ctx.enter_context(tc.tile_pool(name="pf", bufs=1, space="PSUM"))

    # resident inputs
    xT_s = consts.tile([81, 3, C, BL], F16)
    nc.sync.dma_start(out=xT_s, in_=xT)
    Wb_s = consts.tile([81, 3, C, R * S], F16)
    nc.sync.dma_start(out=Wb_s, in_=Wb)
    WoT_s = consts.tile([81, C, O], F16)
    nc.sync.dma_start(out=WoT_s, in_=WoT)
    ident = consts.tile([128, 128], F16)
    make_identity(nc, ident)
    tiny_b = consts.tile([128, 1], F32)
    nc.vector.memset(tiny_b, 1e-30)

    for t in range(NT):
        bsl = slice(t * 128, (t + 1) * 128)
        zbuf = sb.tile([128, C * S], F32, tag="zbuf")
        # ---- phase A: projections + 3-way product ----
        mbuf = sb.tile([128, C, R * S], F16, tag="mbuf")
        for c in range(C):
            proj = []
            for m in range(3):
                p = pp.tile([128, R * S], F32, tag="proj")
                nc.tensor.matmul(
                    p, lhsT=xT_s[:, m, c, bsl], rhs=Wb_s[:, m, c, :],
                    start=True, stop=True,
                )
                proj.append(p)
            # DVE has a single PSUM read port: at most one PSUM operand per
            # tensor_tensor. Evacuate proj0 PSUM->SBUF on ScalarE first.
            p0c = sb.tile([128, R * S], F32, tag="p0c")
            nc.scalar.copy(p0c, proj[0])
            m01 = sb.tile([128, R * S], F32, tag="m01")
            nc.vector.tensor_mul(m01, p0c, proj[1])
            nc.vector.tensor_mul(mbuf[:, c, :], m01, proj[2])
        # rank reduce on GpSimd: z[c,s] = sum_r mbuf[c,(s,r)] (4 big adds)
        mbv = mbuf.rearrange("p c (s r) -> p c s r", r=R)
        tr1 = sb.tile([128, C, S], F16, tag="tr1")
        tr2 = sb.tile([128, C, S], F16, tag="tr2")
        nc.gpsimd.tensor_add(tr1, mbv[:, :, :, 0], mbv[:, :, :, 1])
        nc.gpsimd.tensor_add(tr2, mbv[:, :, :, 2], mbv[:, :, :, 3])
        nc.gpsimd.tensor_add(tr1, tr1, tr2)
        zbv = zbuf.rearrange("p (c s) -> p c s", s=S)
        nc.gpsimd.tensor_add(zbv, tr1, mbv[:, :, :, 4])
        # ---- phase B: signed sqrt + L2 normalize ----
        # z' = sign(z)*sqrt(|z|); ||z'||^2 = sum_s |z_s|; g = rsqrt(sum|z|)
        abuf = sb.tile([128, C * S], F32, tag="abuf")
        nc.scalar.activation(out=abuf, in_=zbuf, func=AF.Abs)
        sqb = sb.tile([128, C * S], F32, tag="sqb")
        nc.scalar.activation(out=sqb, in_=abuf, func=AF.Sqrt)
        sgb = sb.tile([128, C * S], F32, tag="sgb")
        nc.scalar.activation(out=sgb, in_=zbuf, func=AF.Sign)
        sa = sb.tile([128, C], F32, tag="sa")
        nc.vector.tensor_reduce(
            out=sa, in_=zbv, axis=mybir.AxisListType.X, op=ALU.add,
            apply_absolute_value=True,
        )
        rsa = sb.tile([128, C], F32, tag="rsa")
        nc.vector.reciprocal(rsa, sa)
        g = sb.tile([128, C], F32, tag="g")
        nc.scalar.activation(out=g, in_=rsa, func=AF.Sqrt)  # g = rsqrt(sum|z|)
        znb = sb.tile([128, C * S], F16, tag="znb")
        for c in range(C):
            cs = slice(c * S, (c + 1) * S)
            nc.vector.scalar_tensor_tensor(
                out=znb[:, cs], in0=sqb[:, cs], scalar=g[:, c:c + 1],
                in1=sgb[:, cs], op0=ALU.mult, op1=ALU.mult,
            )
        # ---- phase C: heads ----
        expb = sb.tile([128, C, O], F32, tag="expb")
        fin_ps = pf.tile([128, O], F32, tag="finps")
        for c in range(C):
            zT_ps = pt.tile([S, 128], F16, tag="zT")
            nc.tensor.transpose(zT_ps, znb[:, c * S:(c + 1) * S], ident)
            zTs = sb.tile([81, 128], F16, tag="zTs")
            nc.gpsimd.memset(zTs, 1.0)  # row 80 stays 1.0 (bias row)
            nc.scalar.copy(zTs[:S, :], zT_ps)
            P_ps = ph.tile([128, O], F32, tag="P")
            nc.tensor.matmul(
                P_ps, lhsT=zTs, rhs=WoT_s[:, c, :], start=True, stop=True,
            )
            # accumulate final logits; bias row only once (c == 0)
            k = 81 if c == 0 else S
            nc.tensor.matmul(
                fin_ps, lhsT=zTs[:k, :], rhs=WoT_s[:k, c, :],
                start=(c == 0), stop=(c == C - 1), skip_group_check=True,
            )
            nc.scalar.activation(out=expb[:, c, :], in_=P_ps, func=AF.Exp)
        den = sb.tile([128, C], F32, tag="den")
        nc.vector.tensor_reduce(
            out=den, in_=expb, axis=mybir.AxisListType.X, op=ALU.add,
        )
        rden = sb.tile([128, C], F32, tag="rden")
        nc.vector.reciprocal(rden, den)
        outc = sb.tile([128, C * O], F32, tag="outc")
        rdb = bass.AP(
            tensor=rden.tensor, offset=rden.offset,
            ap=[rden.ap[0], [1, C], [0, O]],
        )
        nc.vector.tensor_mul(
            outc.rearrange("p (c o) -> p c o", o=O), expb, rdb,
        )
        nc.sync.dma_start(out=chk[bsl, :], in_=outc)
        fexp = sb.tile([128, O], F32, tag="fexp")
        nc.scalar.activation(out=fexp, in_=fin_ps, func=AF.Exp)
        fden = sb.tile([128, 1], F32, tag="fden")
        nc.vector.tensor_reduce(
            out=fden, in_=fexp, axis=mybir.AxisListType.X, op=ALU.add,
        )
        rfden = sb.tile([128, 1], F32, tag="rfden")
        nc.vector.reciprocal(rfden, fden)
        outf = sb.tile([128, O], F32, tag="outf")
        nc.vector.tensor_scalar_mul(outf, fexp, rfden)
        nc.sync.dma_start(out=fin[bsl, :], in_=outf)


def build():
    global _prog
    if _prog is not None:
        return _prog
    nc = bacc.Bacc("TRN2", target_bir_lowering=False, debug=False)
    from contextlib import ExitStack

    with tile.TileContext(nc) as tc, ExitStack() as ctx:
        _emit(nc, tc, ctx)
    nc.compile()
    _prog = nc
    return nc


def _prep_inputs(x0, x1, x2, W, b, W_out, b_out):
    """Host-side shard + layout prep. Returns per-core input dicts."""
    xs = np.stack([x0, x1, x2]).astype(np.float32)       # [3, B, MM]
    src = xs.reshape(3, NCORES, BL, C, S)
    xTc = np.empty((NCORES, 81, 3, C, BL), np.float16)
    xTc[:, :S] = src.transpose(1, 4, 0, 3, 2)            # [core][i][m][c][u]
    xTc[:, S:] = 1.0

    W5 = W.reshape(3, C, R, S, S)
    Wb_a = np.empty((81, 3, C, R * S), np.float16)
    Wb_a[:S] = W5.transpose(4, 0, 1, 3, 2).reshape(S, 3, C, R * S)
    Wb_a[S] = b.reshape(3, C, R, S).transpose(0, 1, 3, 2).reshape(3, C, R * S)

    WoT_a = np.empty((81, C, O), np.float16)
    WoT_a[:S] = W_out.reshape(O, C, S).transpose(2, 1, 0)
    WoT_a[S] = b_out[None, :]

    return [
        {"xT": xTc[i], "Wb": Wb_a, "WoT": WoT_a} for i in range(NCORES)
    ]


def run(x0, x1, x2, W, b, W_out, b_out, trace=False):
    nc = build()
    in_maps = _prep_inputs(
        np.asarray(x0), np.asarray(x1), np.asarray(x2), np.asarray(W),
        np.asarray(b), np.asarray(W_out), np.asarray(b_out),
    )
    res = run_bass_kernel_spmd(nc, in_maps, core_ids=list(range(NCORES)), trace=trace)
    final = np.concatenate([r["fin"] for r in res.results], axis=0)
    chunks = np.concatenate(
        [r["chk"].reshape(BL, C, O) for r in res.results], axis=0
    )
    return (final, chunks), res


def kernel(x0, x1, x2, W, b, W_out, b_out):
    (final, chunks), _ = run(x0, x1, x2, W, b, W_out, b_out, trace=False)
    return final, chunks


# revision 12
# speedup vs baseline: 1.2351x; 1.2351x over previous
"""Trainium2 Bass kernel for nn_BlockTrainerBlend (8-core data parallel).

Math (per batch row):
  split x0/x1/x2 into C=20 chunks of S=80; per (modality m, chunk c):
  proj = x_chunk @ W[m,c]^T + b[m,c]  -> [R*S=400]
  m = proj0*proj1*proj2; z = sum over r -> [80]
  z' = signed-sqrt(z); z_norm = z'/max(||z'||, eps)
  chunk_logits[c] = z_norm[c] @ Wo_c^T + b_out; chunks_out = softmax
  final = softmax(z_flat @ W_out^T + b_out)

Sharding: batch (2048) split 8 ways -> 256 rows/core, two 128-row tiles.
Weights replicated. All matmul operands pre-transposed/cast to fp16 on host,
with a ones-row appended so biases ride inside the matmuls (K=81).
"""
import numpy as np

import concourse.bacc as bacc
import concourse.bass as bass
import concourse.tile as tile
from concourse import mybir
from concourse.bass_utils import run_bass_kernel_spmd
from concourse.masks import make_identity

B, MM, C, S, R, O = 2048, 1600, 20, 80, 5, 27
NCORES = 8
BL = B // NCORES          # 256 rows per core
NT = BL // 128            # 2 batch-tiles per core

F32 = mybir.dt.float32
F16 = mybir.dt.float16
AF = mybir.ActivationFunctionType
ALU = mybir.AluOpType

_prog = None  # cached compiled Bass program


def _emit(nc, tc, ctx):
    xT = nc.dram_tensor("xT", [81, 3, C, BL], F16, kind="ExternalInput").ap()
    Wb = nc.dram_tensor("Wb", [81, 3, C, R * S], F16, kind="ExternalInput").ap()
    WoT = nc.dram_tensor("WoT", [81, C, O], F16, kind="ExternalInput").ap()
    fin = nc.dram_tensor("fin", [BL, O], F32, kind="ExternalOutput").ap()
    chk = nc.dram_tensor("chk", [BL, C * O], F32, kind="ExternalOutput").ap()

    consts = ctx.enter_context(tc.tile_pool(name="consts", bufs=1))
    sb = ctx.enter_context(tc.tile_pool(name="sb", bufs=2))
    pp = ctx.enter_context(tc.tile_pool(name="pp", bufs=4, space="PSUM"))
    pt = ctx.enter_context(tc.tile_pool(name="pt", bufs=2, space="PSUM"))
    ph = ctx.enter_context(tc.tile_pool(name="ph", bufs=1, space="PSUM"))
    pf = ctx.enter_context(tc.tile_pool(name="pf", bufs=1, space="PSUM"))

    # resident inputs — chunked DMAs so chunk-c compute starts as soon as its
    # slices land rather than waiting for the whole 6.5MB load
    WoT_s = consts.tile([81, C, O], F16)
    nc.sync.dma_start(out=WoT_s, in_=WoT)
    ident = consts.tile([128, 128], F16)
    make_identity(nc, ident)
    xT_s = consts.tile([81, 3, C, BL], F16)
    Wb_s = consts.tile([81, 3, C, R * S], F16)
    for c in range(C):
        nc.sync.dma_start(out=xT_s[:, :, c, :], in_=xT[:, :, c, :])
        nc.sync.dma_start(out=Wb_s[:, :, c, :], in_=Wb[:, :, c, :])

    for t in range(NT):
        bsl = slice(t * 128, (t + 1) * 128)
        zbuf = sb.tile([128, C * S], F32, tag="zbuf")
        # ---- phase A: projections + 3-way product ----
        mbuf = sb.tile([128, C, R * S], F16, tag="mbuf")
        for c in range(C):
            proj = []
            for m in range(3):
                p = pp.tile([128, R * S], F32, tag="proj")
                nc.tensor.matmul(
                    p, lhsT=xT_s[:, m, c, bsl], rhs=Wb_s[:, m, c, :],
                    start=True, stop=True,
                )
                proj.append(p)
            # DVE has a single PSUM read port: at most one PSUM operand per
            # tensor_tensor. Evacuate proj0 PSUM->SBUF on ScalarE first.
            p0c = sb.tile([128, R * S], F16, tag="p0c")
            nc.scalar.copy(p0c, proj[0])
            m01 = sb.tile([128, R * S], F32, tag="m01")
            nc.vector.tensor_mul(m01, p0c, proj[1])
            nc.vector.tensor_mul(mbuf[:, c, :], m01, proj[2])
            # rank reduce on GpSimd in groups of 5 chunks (pipelines with
            # the next group's matmuls instead of one big tail barrier)
            if c % 5 == 4:
                g0 = c - 4
                csl = slice(g0, c + 1)
                mbv = mbuf.rearrange("p c (s r) -> p c s r", r=R)
                tr1 = sb.tile([128, 5, S], F16, tag="tr1")
                tr2 = sb.tile([128, 5, S], F16, tag="tr2")
                zbv = zbuf.rearrange("p (c s) -> p c s", s=S)
                nc.gpsimd.tensor_add(tr1, mbv[:, csl, :, 0], mbv[:, csl, :, 1])
                nc.gpsimd.tensor_add(tr2, mbv[:, csl, :, 2], mbv[:, csl, :, 3])
                nc.gpsimd.tensor_add(tr1, tr1, tr2)
                nc.gpsimd.tensor_add(zbv[:, csl, :], tr1, mbv[:, csl, :, 4])
        # ---- phase B: signed sqrt + L2 normalize ----
        # z' = sign(z)*sqrt(|z|); ||z'||^2 = sum_s |z_s|; g = rsqrt(sum|z|)
        abuf = sb.tile([128, C * S], F32, tag="abuf")
        nc.scalar.activation(out=abuf, in_=zbuf, func=AF.Abs)
        sqb = sb.tile([128, C * S], F32, tag="sqb")
        nc.scalar.activation(out=sqb, in_=abuf, func=AF.Sqrt)
        sgb = sb.tile([128, C * S], F32, tag="sgb")
        nc.scalar.activation(out=sgb, in_=zbuf, func=AF.Sign)
        sa = sb.tile([128, C], F32, tag="sa")
        nc.vector.tensor_reduce(
            out=sa, in_=zbv, axis=mybir.AxisListType.X, op=ALU.add,
            apply_absolute_value=True,
        )
        rsa = sb.tile([128, C], F32, tag="rsa")
        nc.vector.reciprocal(rsa, sa)
        g = sb.tile([128, C], F32, tag="g")
        nc.scalar.activation(out=g, in_=rsa, func=AF.Sqrt)  # g = rsqrt(sum|z|)
        znb = sb.tile([128, C * S], F16, tag="znb")
        for c in range(C):
            cs = slice(c * S, (c + 1) * S)
            nc.vector.scalar_tensor_tensor(
                out=znb[:, cs], in0=sqb[:, cs], scalar=g[:, c:c + 1],
                in1=sgb[:, cs], op0=ALU.mult, op1=ALU.mult,
            )
        # ---- phase C: heads ----
        expb = sb.tile([128, C, O], F32, tag="expb")
        fin_ps = pf.tile([128, O], F32, tag="finps")
        for c in range(C):
            zT_ps = pt.tile([S, 128], F16, tag="zT")
            nc.tensor.transpose(zT_ps, znb[:, c * S:(c + 1) * S], ident)
            zTs = sb.tile([81, 128], F16, tag="zTs")
            nc.gpsimd.memset(zTs, 1.0)  # row 80 stays 1.0 (bias row)
            nc.scalar.copy(zTs[:S, :], zT_ps)
            P_ps = ph.tile([128, O], F32, tag="P")
            nc.tensor.matmul(
                P_ps, lhsT=zTs, rhs=WoT_s[:, c, :], start=True, stop=True,
            )
            # accumulate final logits; bias row only once (c == 0)
            k = 81 if c == 0 else S
            nc.tensor.matmul(
                fin_ps, lhsT=zTs[:k, :], rhs=WoT_s[:k, c, :],
                start=(c == 0), stop=(c == C - 1), skip_group_check=True,
            )
            nc.scalar.activation(out=expb[:, c, :], in_=P_ps, func=AF.Exp)
        den = sb.tile([128, C], F32, tag="den")
        nc.vector.tensor_reduce(
            out=den, in_=expb, axis=mybir.AxisListType.X, op=ALU.add,
        )
        rden = sb.tile([128, C], F32, tag="rden")
        nc.vector.reciprocal(rden, den)
        outc = sb.tile([128, C * O], F32, tag="outc")
        rdb = bass.AP(
            tensor=rden.tensor, offset=rden.offset,
            ap=[rden.ap[0], [1, C], [0, O]],
        )
        nc.vector.tensor_mul(
            outc.rearrange("p (c o) -> p c o", o=O), expb, rdb,
        )
        nc.sync.dma_start(out=chk[bsl, :], in_=outc)
        fexp = sb.tile([128, O], F32, tag="fexp")
        nc.scalar.activation(out=fexp, in_=fin_ps, func=AF.Exp)
        fden = sb.tile([128, 1], F32, tag="fden")
        nc.vector.tensor_reduce(
            out=fden, in_=fexp, axis=mybir.AxisListType.X, op=ALU.add,
        )
        rfden = sb.tile([128, 1], F32, tag="rfden")
        nc.vector.reciprocal(rfden, fden)
        outf = sb.tile([128, O], F32, tag="outf")
        nc.vector.tensor_scalar_mul(outf, fexp, rfden)
        nc.sync.dma_start(out=fin[bsl, :], in_=outf)


def build():
    global _prog
    if _prog is not None:
        return _prog
    nc = bacc.Bacc("TRN2", target_bir_lowering=False, debug=False)
    from contextlib import ExitStack

    with tile.TileContext(nc) as tc, ExitStack() as ctx:
        _emit(nc, tc, ctx)
    nc.compile()
    _prog = nc
    return nc


def _prep_inputs(x0, x1, x2, W, b, W_out, b_out):
    """Host-side shard + layout prep. Returns per-core input dicts."""
    xs = np.stack([x0, x1, x2]).astype(np.float32)       # [3, B, MM]
    src = xs.reshape(3, NCORES, BL, C, S)
    xTc = np.empty((NCORES, 81, 3, C, BL), np.float16)
    xTc[:, :S] = src.transpose(1, 4, 0, 3, 2)            # [core][i][m][c][u]
    xTc[:, S:] = 1.0

    W5 = W.reshape(3, C, R, S, S)
    Wb_a = np.empty((81, 3, C, R * S), np.float16)
    Wb_a[:S] = W5.transpose(4, 0, 1, 3, 2).reshape(S, 3, C, R * S)
    Wb_a[S] = b.reshape(3, C, R, S).transpose(0, 1, 3, 2).reshape(3, C, R * S)

    WoT_a = np.empty((81, C, O), np.float16)
    WoT_a[:S] = W_out.reshape(O, C, S).transpose(2, 1, 0)
    WoT_a[S] = b_out[None, :]

    return [
        {"xT": xTc[i], "Wb": Wb_a, "WoT": WoT_a} for i in range(NCORES)
    ]


def run(x0, x1, x2, W, b, W_out, b_out, trace=False):
    nc = build()
    in_maps = _prep_inputs(
        np.asarray(x0), np.asarray(x1), np.asarray(x2), np.asarray(W),
        np.asarray(b), np.asarray(W_out), np.asarray(b_out),
    )
    res = run_bass_kernel_spmd(nc, in_maps, core_ids=list(range(NCORES)), trace=trace)
    final = np.concatenate([r["fin"] for r in res.results], axis=0)
    chunks = np.concatenate(
        [r["chk"].reshape(BL, C, O) for r in res.results], axis=0
    )
    return (final, chunks), res


def kernel(x0, x1, x2, W, b, W_out, b_out):
    (final, chunks), _ = run(x0, x1, x2, W, b, W_out, b_out, trace=False)
    return final, chunks


# revision 15
# speedup vs baseline: 1.2402x; 1.0041x over previous
"""Trainium2 Bass kernel for nn_BlockTrainerBlend (8-core data parallel).

Math (per batch row):
  split x0/x1/x2 into C=20 chunks of S=80; per (modality m, chunk c):
  proj = x_chunk @ W[m,c]^T + b[m,c]  -> [R*S=400]
  m = proj0*proj1*proj2; z = sum over r -> [80]
  z' = signed-sqrt(z); z_norm = z'/max(||z'||, eps)
  chunk_logits[c] = z_norm[c] @ Wo_c^T + b_out; chunks_out = softmax
  final = softmax(z_flat @ W_out^T + b_out)

Sharding: batch (2048) split 8 ways -> 256 rows/core, two 128-row tiles.
Weights replicated. All matmul operands pre-transposed/cast to fp16 on host,
with a ones-row appended so biases ride inside the matmuls (K=81).
"""
import numpy as np

import concourse.bacc as bacc
import concourse.bass as bass
import concourse.tile as tile
from concourse import mybir
from concourse.bass_utils import run_bass_kernel_spmd
from concourse.masks import make_identity

B, MM, C, S, R, O = 2048, 1600, 20, 80, 5, 27
NCORES = 8
BL = B // NCORES          # 256 rows per core
NT = BL // 128            # 2 batch-tiles per core

F32 = mybir.dt.float32
F16 = mybir.dt.float16
AF = mybir.ActivationFunctionType
ALU = mybir.AluOpType

_prog = None  # cached compiled Bass program


def _emit(nc, tc, ctx):
    xT = nc.dram_tensor("xT", [81, 3, C, BL], F16, kind="ExternalInput").ap()
    Wb = nc.dram_tensor("Wb", [81, 3, C, R * S], F16, kind="ExternalInput").ap()
    WoT = nc.dram_tensor("WoT", [81, C, O], F16, kind="ExternalInput").ap()
    fin = nc.dram_tensor("fin", [BL, O], F32, kind="ExternalOutput").ap()
    chk = nc.dram_tensor("chk", [BL, C * O], F32, kind="ExternalOutput").ap()

    consts = ctx.enter_context(tc.tile_pool(name="consts", bufs=1))
    sb = ctx.enter_context(tc.tile_pool(name="sb", bufs=2))
    pp = ctx.enter_context(tc.tile_pool(name="pp", bufs=5, space="PSUM"))
    pt = ctx.enter_context(tc.tile_pool(name="pt", bufs=1, space="PSUM"))
    ph = ctx.enter_context(tc.tile_pool(name="ph", bufs=1, space="PSUM"))
    pf = ctx.enter_context(tc.tile_pool(name="pf", bufs=1, space="PSUM"))

    # resident inputs — chunked DMAs so chunk-c compute starts as soon as its
    # slices land rather than waiting for the whole 6.5MB load
    WoT_s = consts.tile([81, C, O], F16)
    nc.sync.dma_start(out=WoT_s, in_=WoT)
    ident = consts.tile([128, 128], F16)
    make_identity(nc, ident)
    xT_s = consts.tile([81, 3, C, BL], F16)
    Wb_s = consts.tile([81, 3, C, R * S], F16)
    for c in range(C):
        nc.sync.dma_start(out=xT_s[:, :, c, :], in_=xT[:, :, c, :])
        nc.sync.dma_start(out=Wb_s[:, :, c, :], in_=Wb[:, :, c, :])

    for t in range(NT):
        bsl = slice(t * 128, (t + 1) * 128)
        zbuf = sb.tile([128, C * S], F32, tag="zbuf")
        # ---- phase A: projections + 3-way product ----
        mbuf = sb.tile([128, C, R * S], F16, tag="mbuf")
        for c in range(C):
            proj = []
            for m in range(3):
                p = pp.tile([128, R * S], F32, tag="proj")
                nc.tensor.matmul(
                    p, lhsT=xT_s[:, m, c, bsl], rhs=Wb_s[:, m, c, :],
                    start=True, stop=True,
                )
                proj.append(p)
            # DVE has a single PSUM read port: at most one PSUM operand per
            # tensor_tensor. Evacuate proj0 PSUM->SBUF on ScalarE first.
            p0c = sb.tile([128, R * S], F16, tag="p0c")
            nc.scalar.copy(p0c, proj[0])
            m01 = sb.tile([128, R * S], F32, tag="m01")
            nc.vector.tensor_mul(m01, p0c, proj[1])
            nc.vector.tensor_mul(mbuf[:, c, :], m01, proj[2])
            # rank reduce on GpSimd in groups of 5 chunks (pipelines with
            # the next group's matmuls instead of one big tail barrier)
            if c % 5 == 4:
                g0 = c - 4
                csl = slice(g0, c + 1)
                mbv = mbuf.rearrange("p c (s r) -> p c s r", r=R)
                tr1 = sb.tile([128, 5, S], F16, tag="tr1")
                tr2 = sb.tile([128, 5, S], F16, tag="tr2")
                zbv = zbuf.rearrange("p (c s) -> p c s", s=S)
                nc.gpsimd.tensor_add(tr1, mbv[:, csl, :, 0], mbv[:, csl, :, 1])
                nc.gpsimd.tensor_add(tr2, mbv[:, csl, :, 2], mbv[:, csl, :, 3])
                nc.gpsimd.tensor_add(tr1, tr1, tr2)
                nc.gpsimd.tensor_add(zbv[:, csl, :], tr1, mbv[:, csl, :, 4])
        # ---- phase B: signed sqrt + L2 normalize ----
        # z' = sign(z)*sqrt(|z|); ||z'||^2 = sum_s |z_s|; g = rsqrt(sum|z|)
        abuf = sb.tile([128, C * S], F32, tag="abuf")
        nc.scalar.activation(out=abuf, in_=zbuf, func=AF.Abs)
        sqb = sb.tile([128, C * S], F16, tag="sqb")
        nc.scalar.activation(out=sqb, in_=abuf, func=AF.Sqrt)
        sgb = sb.tile([128, C * S], F16, tag="sgb")
        nc.scalar.activation(out=sgb, in_=zbuf, func=AF.Sign)
        sa = sb.tile([128, C], F32, tag="sa")
        nc.vector.tensor_reduce(
            out=sa, in_=zbv, axis=mybir.AxisListType.X, op=ALU.add,
            apply_absolute_value=True,
        )
        rsa = sb.tile([128, C], F32, tag="rsa")
        nc.vector.reciprocal(rsa, sa)
        g = sb.tile([128, C], F16, tag="g")
        nc.scalar.activation(out=g, in_=rsa, func=AF.Sqrt)  # g = rsqrt(sum|z|)
        zf = sb.tile([128, C * S], F16, tag="zf")
        nc.vector.tensor_mul(zf, sqb, sgb)
        znb = sb.tile([128, C * S], F16, tag="znb")
        gb = bass.AP(
            tensor=g.tensor, offset=g.offset, ap=[g.ap[0], [1, C], [0, S]],
        )
        nc.vector.tensor_mul(
            znb.rearrange("p (c s) -> p c s", s=S),
            zf.rearrange("p (c s) -> p c s", s=S), gb,
        )
        # ---- phase C: heads ----
        # 5 chunks share one PSUM bank for their logits -> one Exp per group
        expb = sb.tile([128, C, O], F32, tag="expb")
        fin_ps = pf.tile([128, O], F32, tag="finps")
        for c5 in range(C // 5):
            P5_ps = ph.tile([128, 5, O], F32, tag="P")
            for j in range(5):
                c = c5 * 5 + j
                zT_ps = pt.tile([S, 128], F16, tag="zT")
                nc.tensor.transpose(zT_ps, znb[:, c * S:(c + 1) * S], ident)
                zTs = sb.tile([81, 128], F16, tag="zTs")
                nc.gpsimd.memset(zTs, 1.0)  # row 80 stays 1.0 (bias row)
                nc.scalar.copy(zTs[:S, :], zT_ps)
                nc.tensor.matmul(
                    P5_ps[:, j, :], lhsT=zTs, rhs=WoT_s[:, c, :],
                    start=True, stop=True,
                )
                # accumulate final logits; bias row only once (c == 0)
                k = 81 if c == 0 else S
                nc.tensor.matmul(
                    fin_ps, lhsT=zTs[:k, :], rhs=WoT_s[:k, c, :],
                    start=(c == 0), stop=(c == C - 1), skip_group_check=True,
                )
            nc.scalar.activation(
                out=expb[:, c5 * 5:(c5 + 1) * 5, :], in_=P5_ps, func=AF.Exp,
            )
        den = sb.tile([128, C], F32, tag="den")
        nc.vector.tensor_reduce(
            out=den, in_=expb, axis=mybir.AxisListType.X, op=ALU.add,
        )
        rden = sb.tile([128, C], F32, tag="rden")
        nc.vector.reciprocal(rden, den)
        outc = sb.tile([128, C * O], F32, tag="outc")
        rdb = bass.AP(
            tensor=rden.tensor, offset=rden.offset,
            ap=[rden.ap[0], [1, C], [0, O]],
        )
        nc.vector.tensor_mul(
            outc.rearrange("p (c o) -> p c o", o=O), expb, rdb,
        )
        nc.sync.dma_start(out=chk[bsl, :], in_=outc)
        fexp = sb.tile([128, O], F32, tag="fexp")
        nc.scalar.activation(out=fexp, in_=fin_ps, func=AF.Exp)
        fden = sb.tile([128, 1], F32, tag="fden")
        nc.vector.tensor_reduce(
            out=fden, in_=fexp, axis=mybir.AxisListType.X, op=ALU.add,
        )
        rfden = sb.tile([128, 1], F32, tag="rfden")
        nc.vector.reciprocal(rfden, fden)
        outf = sb.tile([128, O], F32, tag="outf")
        nc.vector.tensor_scalar_mul(outf, fexp, rfden)
        nc.sync.dma_start(out=fin[bsl, :], in_=outf)


def build():
    global _prog
    if _prog is not None:
        return _prog
    nc = bacc.Bacc("TRN2", target_bir_lowering=False, debug=False)
    from contextlib import ExitStack

    with tile.TileContext(nc) as tc, ExitStack() as ctx:
        _emit(nc, tc, ctx)
    nc.compile()
    _prog = nc
    return nc


def _prep_inputs(x0, x1, x2, W, b, W_out, b_out):
    """Host-side shard + layout prep. Returns per-core input dicts."""
    xs = np.stack([x0, x1, x2]).astype(np.float32)       # [3, B, MM]
    src = xs.reshape(3, NCORES, BL, C, S)
    xTc = np.empty((NCORES, 81, 3, C, BL), np.float16)
    xTc[:, :S] = src.transpose(1, 4, 0, 3, 2)            # [core][i][m][c][u]
    xTc[:, S:] = 1.0

    W5 = W.reshape(3, C, R, S, S)
    Wb_a = np.empty((81, 3, C, R * S), np.float16)
    Wb_a[:S] = W5.transpose(4, 0, 1, 3, 2).reshape(S, 3, C, R * S)
    Wb_a[S] = b.reshape(3, C, R, S).transpose(0, 1, 3, 2).reshape(3, C, R * S)

    WoT_a = np.empty((81, C, O), np.float16)
    WoT_a[:S] = W_out.reshape(O, C, S).transpose(2, 1, 0)
    WoT_a[S] = b_out[None, :]

    return [
        {"xT": xTc[i], "Wb": Wb_a, "WoT": WoT_a} for i in range(NCORES)
    ]


def run(x0, x1, x2, W, b, W_out, b_out, trace=False):
    nc = build()
    in_maps = _prep_inputs(
        np.asarray(x0), np.asarray(x1), np.asarray(x2), np.asarray(W),
        np.asarray(b), np.asarray(W_out), np.asarray(b_out),
    )
    res = run_bass_kernel_spmd(nc, in_maps, core_ids=list(range(NCORES)), trace=trace)
    final = np.concatenate([r["fin"] for r in res.results], axis=0)
    chunks = np.concatenate(
        [r["chk"].reshape(BL, C, O) for r in res.results], axis=0
    )
    return (final, chunks), res


def kernel(x0, x1, x2, W, b, W_out, b_out):
    (final, chunks), _ = run(x0, x1, x2, W, b, W_out, b_out, trace=False)
    return final, chunks


# revision 17
# speedup vs baseline: 1.2555x; 1.0124x over previous
"""Trainium2 Bass kernel for nn_BlockTrainerBlend (8-core data parallel).

Math (per batch row):
  split x0/x1/x2 into C=20 chunks of S=80; per (modality m, chunk c):
  proj = x_chunk @ W[m,c]^T + b[m,c]  -> [R*S=400]
  m = proj0*proj1*proj2; z = sum over r -> [80]
  z' = signed-sqrt(z); z_norm = z'/max(||z'||, eps)
  chunk_logits[c] = z_norm[c] @ Wo_c^T + b_out; chunks_out = softmax
  final = softmax(z_flat @ W_out^T + b_out)

Sharding: batch (2048) split 8 ways -> 256 rows/core, two 128-row tiles.
Weights replicated. All matmul operands pre-transposed/cast to fp16 on host,
with a ones-row appended so biases ride inside the matmuls (K=81).
"""
import numpy as np

import concourse.bacc as bacc
import concourse.bass as bass
import concourse.tile as tile
from concourse import mybir
from concourse.bass_utils import run_bass_kernel_spmd
from concourse.masks import make_identity

B, MM, C, S, R, O = 2048, 1600, 20, 80, 5, 27
NCORES = 8
BL = B // NCORES          # 256 rows per core
NT = BL // 128            # 2 batch-tiles per core

F32 = mybir.dt.float32
F16 = mybir.dt.float16
AF = mybir.ActivationFunctionType
ALU = mybir.AluOpType

_prog = None  # cached compiled Bass program


def _emit(nc, tc, ctx):
    xT = nc.dram_tensor("xT", [81, 3, C, BL], F16, kind="ExternalInput").ap()
    Wb = nc.dram_tensor("Wb", [81, 3, C, R * S], F16, kind="ExternalInput").ap()
    WoT = nc.dram_tensor("WoT", [81, C, O], F16, kind="ExternalInput").ap()
    fin = nc.dram_tensor("fin", [BL, O], F32, kind="ExternalOutput").ap()
    chk = nc.dram_tensor("chk", [BL, C * O], F32, kind="ExternalOutput").ap()

    consts = ctx.enter_context(tc.tile_pool(name="consts", bufs=1))
    sb = ctx.enter_context(tc.tile_pool(name="sb", bufs=2))
    pp0 = ctx.enter_context(tc.tile_pool(name="pp0", bufs=2, space="PSUM"))
    pp = ctx.enter_context(tc.tile_pool(name="pp", bufs=3, space="PSUM"))
    pt = ctx.enter_context(tc.tile_pool(name="pt", bufs=1, space="PSUM"))
    ph = ctx.enter_context(tc.tile_pool(name="ph", bufs=1, space="PSUM"))
    pf = ctx.enter_context(tc.tile_pool(name="pf", bufs=1, space="PSUM"))

    # resident inputs — chunked DMAs so chunk-c compute starts as soon as its
    # slices land rather than waiting for the whole 6.5MB load
    WoT_s = consts.tile([81, C, O], F16)
    nc.sync.dma_start(out=WoT_s, in_=WoT)
    ident = consts.tile([128, 128], F16)
    make_identity(nc, ident)
    xT_s = consts.tile([81, 3, C, BL], F16)
    Wb_s = consts.tile([81, 3, C, R * S], F16)
    for c in range(C):
        nc.sync.dma_start(out=xT_s[:, :, c, :], in_=xT[:, :, c, :])
        nc.sync.dma_start(out=Wb_s[:, :, c, :], in_=Wb[:, :, c, :])

    for t in range(NT):
        bsl = slice(t * 128, (t + 1) * 128)
        zbuf = sb.tile([128, C * S], F32, tag="zbuf")
        # ---- phase A: projections + 3-way product ----
        mbuf = sb.tile([128, C, R * S], F16, tag="mbuf")
        for c in range(C):
            proj = []
            for m in range(3):
                pool_m = pp0 if m == 0 else pp
                p = pool_m.tile([128, R * S], F32, tag="proj0" if m == 0 else "proj")
                nc.tensor.matmul(
                    p, lhsT=xT_s[:, m, c, bsl], rhs=Wb_s[:, m, c, :],
                    start=True, stop=True,
                )
                proj.append(p)
            # DVE has a single PSUM read port: at most one PSUM operand per
            # tensor_tensor. Evacuate proj0 PSUM->SBUF on ScalarE first.
            p0c = sb.tile([128, R * S], F16, tag="p0c")
            nc.scalar.copy(p0c, proj[0])
            m01 = sb.tile([128, R * S], F32, tag="m01")
            nc.vector.tensor_mul(m01, p0c, proj[1])
            nc.vector.tensor_mul(mbuf[:, c, :], m01, proj[2])
            # rank reduce on GpSimd in groups of 5 chunks (pipelines with
            # the next group's matmuls instead of one big tail barrier)
            if c % 5 == 4:
                g0 = c - 4
                csl = slice(g0, c + 1)
                mbv = mbuf.rearrange("p c (s r) -> p c s r", r=R)
                tr1 = sb.tile([128, 5, S], F16, tag="tr1")
                tr2 = sb.tile([128, 5, S], F16, tag="tr2")
                zbv = zbuf.rearrange("p (c s) -> p c s", s=S)
                nc.gpsimd.tensor_add(tr1, mbv[:, csl, :, 0], mbv[:, csl, :, 1])
                nc.gpsimd.tensor_add(tr2, mbv[:, csl, :, 2], mbv[:, csl, :, 3])
                nc.gpsimd.tensor_add(tr1, tr1, tr2)
                nc.gpsimd.tensor_add(zbv[:, csl, :], tr1, mbv[:, csl, :, 4])
        # ---- phase B: signed sqrt + L2 normalize ----
        # z' = sign(z)*sqrt(|z|); ||z'||^2 = sum_s |z_s|; g = rsqrt(sum|z|)
        abuf = sb.tile([128, C * S], F32, tag="abuf")
        nc.scalar.activation(out=abuf, in_=zbuf, func=AF.Abs)
        sqb = sb.tile([128, C * S], F16, tag="sqb")
        nc.scalar.activation(out=sqb, in_=abuf, func=AF.Sqrt)
        sgb = sb.tile([128, C * S], F16, tag="sgb")
        nc.scalar.activation(out=sgb, in_=zbuf, func=AF.Sign)
        sa = sb.tile([128, C], F32, tag="sa")
        nc.vector.tensor_reduce(
            out=sa, in_=zbv, axis=mybir.AxisListType.X, op=ALU.add,
            apply_absolute_value=True,
        )
        rsa = sb.tile([128, C], F32, tag="rsa")
        nc.vector.reciprocal(rsa, sa)
        g = sb.tile([128, C], F16, tag="g")
        nc.scalar.activation(out=g, in_=rsa, func=AF.Sqrt)  # g = rsqrt(sum|z|)
        zf = sb.tile([128, C * S], F16, tag="zf")
        nc.vector.tensor_mul(zf, sqb, sgb)
        znb = sb.tile([128, C * S], F16, tag="znb")
        gb = bass.AP(
            tensor=g.tensor, offset=g.offset, ap=[g.ap[0], [1, C], [0, S]],
        )
        nc.vector.tensor_mul(
            znb.rearrange("p (c s) -> p c s", s=S),
            zf.rearrange("p (c s) -> p c s", s=S), gb,
        )
        # ---- phase C: heads ----
        # 5 chunks share one PSUM bank for their logits -> one Exp per group
        expb = sb.tile([128, C, O], F32, tag="expb")
        fin_ps = pf.tile([128, O], F32, tag="finps")
        for c5 in range(C // 5):
            P5_ps = ph.tile([128, 5, O], F32, tag="P")
            for j in range(5):
                c = c5 * 5 + j
                zT_ps = pt.tile([S, 128], F16, tag="zT")
                nc.tensor.transpose(zT_ps, znb[:, c * S:(c + 1) * S], ident)
                zTs = sb.tile([81, 128], F16, tag="zTs")
                nc.gpsimd.memset(zTs, 1.0)  # row 80 stays 1.0 (bias row)
                nc.scalar.copy(zTs[:S, :], zT_ps)
                nc.tensor.matmul(
                    P5_ps[:, j, :], lhsT=zTs, rhs=WoT_s[:, c, :],
                    start=True, stop=True,
                )
                # accumulate final logits; bias row only once (c == 0)
                k = 81 if c == 0 else S
                nc.tensor.matmul(
                    fin_ps, lhsT=zTs[:k, :], rhs=WoT_s[:k, c, :],
                    start=(c == 0), stop=(c == C - 1), skip_group_check=True,
                )
            nc.scalar.activation(
                out=expb[:, c5 * 5:(c5 + 1) * 5, :], in_=P5_ps, func=AF.Exp,
            )
        den = sb.tile([128, C], F32, tag="den")
        nc.vector.tensor_reduce(
            out=den, in_=expb, axis=mybir.AxisListType.X, op=ALU.add,
        )
        rden = sb.tile([128, C], F32, tag="rden")
        nc.vector.reciprocal(rden, den)
        outc = sb.tile([128, C * O], F32, tag="outc")
        rdb = bass.AP(
            tensor=rden.tensor, offset=rden.offset,
            ap=[rden.ap[0], [1, C], [0, O]],
        )
        nc.vector.tensor_mul(
            outc.rearrange("p (c o) -> p c o", o=O), expb, rdb,
        )
        nc.sync.dma_start(out=chk[bsl, :], in_=outc)
        fexp = sb.tile([128, O], F32, tag="fexp")
        nc.scalar.activation(out=fexp, in_=fin_ps, func=AF.Exp)
        fden = sb.tile([128, 1], F32, tag="fden")
        nc.vector.tensor_reduce(
            out=fden, in_=fexp, axis=mybir.AxisListType.X, op=ALU.add,
        )
        rfden = sb.tile([128, 1], F32, tag="rfden")
        nc.vector.reciprocal(rfden, fden)
        outf = sb.tile([128, O], F32, tag="outf")
        nc.vector.tensor_scalar_mul(outf, fexp, rfden)
        nc.sync.dma_start(out=fin[bsl, :], in_=outf)


def build():
    global _prog
    if _prog is not None:
        return _prog
    nc = bacc.Bacc("TRN2", target_bir_lowering=False, debug=False)
    from contextlib import ExitStack

    with tile.TileContext(nc) as tc, ExitStack() as ctx:
        _emit(nc, tc, ctx)
    nc.compile()
    _prog = nc
    return nc


def _prep_inputs(x0, x1, x2, W, b, W_out, b_out):
    """Host-side shard + layout prep. Returns per-core input dicts."""
    xs = np.stack([x0, x1, x2]).astype(np.float32)       # [3, B, MM]
    src = xs.reshape(3, NCORES, BL, C, S)
    xTc = np.empty((NCORES, 81, 3, C, BL), np.float16)
    xTc[:, :S] = src.transpose(1, 4, 0, 3, 2)            # [core][i][m][c][u]
    xTc[:, S:] = 1.0

    W5 = W.reshape(3, C, R, S, S)
    Wb_a = np.empty((81, 3, C, R * S), np.float16)
    Wb_a[:S] = W5.transpose(4, 0, 1, 3, 2).reshape(S, 3, C, R * S)
    Wb_a[S] = b.reshape(3, C, R, S).transpose(0, 1, 3, 2).reshape(3, C, R * S)

    WoT_a = np.empty((81, C, O), np.float16)
    WoT_a[:S] = W_out.reshape(O, C, S).transpose(2, 1, 0)
    WoT_a[S] = b_out[None, :]

    return [
        {"xT": xTc[i], "Wb": Wb_a, "WoT": WoT_a} for i in range(NCORES)
    ]


def run(x0, x1, x2, W, b, W_out, b_out, trace=False):
    nc = build()
    in_maps = _prep_inputs(
        np.asarray(x0), np.asarray(x1), np.asarray(x2), np.asarray(W),
        np.asarray(b), np.asarray(W_out), np.asarray(b_out),
    )
    res = run_bass_kernel_spmd(nc, in_maps, core_ids=list(range(NCORES)), trace=trace)
    final = np.concatenate([r["fin"] for r in res.results], axis=0)
    chunks = np.concatenate(
        [r["chk"].reshape(BL, C, O) for r in res.results], axis=0
    )
    return (final, chunks), res


def kernel(x0, x1, x2, W, b, W_out, b_out):
    (final, chunks), _ = run(x0, x1, x2, W, b, W_out, b_out, trace=False)
    return final, chunks


# revision 21
# speedup vs baseline: 1.2781x; 1.0180x over previous
"""Trainium2 Bass kernel for nn_BlockTrainerBlend (8-core data parallel).

Math (per batch row):
  split x0/x1/x2 into C=20 chunks of S=80; per (modality m, chunk c):
  proj = x_chunk @ W[m,c]^T + b[m,c]  -> [R*S=400]
  m = proj0*proj1*proj2; z = sum over r -> [80]
  z' = signed-sqrt(z); z_norm = z'/max(||z'||, eps)
  chunk_logits[c] = z_norm[c] @ Wo_c^T + b_out; chunks_out = softmax
  final = softmax(z_flat @ W_out^T + b_out)

Sharding: batch (2048) split 8 ways -> 256 rows/core, two 128-row tiles.
Weights replicated. All matmul operands pre-transposed/cast to fp16 on host,
with a ones-row appended so biases ride inside the matmuls (K=81).
"""
import numpy as np

import concourse.bacc as bacc
import concourse.bass as bass
import concourse.tile as tile
from concourse import mybir
from concourse.bass_utils import run_bass_kernel_spmd
from concourse.masks import make_identity

B, MM, C, S, R, O = 2048, 1600, 20, 80, 5, 27
NCORES = 8
BL = B // NCORES          # 256 rows per core
NT = BL // 128            # 2 batch-tiles per core

F32 = mybir.dt.float32
F16 = mybir.dt.float16
AF = mybir.ActivationFunctionType
ALU = mybir.AluOpType

_prog = None  # cached compiled Bass program


def _emit(nc, tc, ctx):
    # chunk-major so each chunk's load is one contiguous 2D DMA pattern
    xT = nc.dram_tensor("xT", [C, 81, 3, BL], F16, kind="ExternalInput").ap()
    Wb = nc.dram_tensor("Wb", [C, 81, 3, R * S], F16, kind="ExternalInput").ap()
    WoT = nc.dram_tensor("WoT", [81, C, O], F16, kind="ExternalInput").ap()
    fin = nc.dram_tensor("fin", [BL, O], F32, kind="ExternalOutput").ap()
    chk = nc.dram_tensor("chk", [BL, C * O], F32, kind="ExternalOutput").ap()

    consts = ctx.enter_context(tc.tile_pool(name="consts", bufs=1))
    sb = ctx.enter_context(tc.tile_pool(name="sb", bufs=2))
    pp0 = ctx.enter_context(tc.tile_pool(name="pp0", bufs=2, space="PSUM"))
    pp = ctx.enter_context(tc.tile_pool(name="pp", bufs=3, space="PSUM"))
    pt = ctx.enter_context(tc.tile_pool(name="pt", bufs=1, space="PSUM"))
    ph = ctx.enter_context(tc.tile_pool(name="ph", bufs=1, space="PSUM"))
    pf = ctx.enter_context(tc.tile_pool(name="pf", bufs=1, space="PSUM"))

    # resident inputs — chunked DMAs so chunk-c compute starts as soon as its
    # slices land rather than waiting for the whole 6.5MB load
    WoT_s = consts.tile([81, C, O], F16)
    nc.sync.dma_start(out=WoT_s, in_=WoT)
    ident = consts.tile([128, 128], F16)
    make_identity(nc, ident)
    xT_s = consts.tile([81, C, 3, BL], F16)
    Wb_s = consts.tile([81, C, 3, R * S], F16)
    for c in range(C):
        nc.sync.dma_start(out=xT_s[:, c, :, :], in_=xT[c])
        nc.sync.dma_start(out=Wb_s[:, c, :, :], in_=Wb[c])

    for t in range(NT):
        bsl = slice(t * 128, (t + 1) * 128)
        zbuf = sb.tile([128, C * S], F32, tag="zbuf")
        # ---- phase A: projections + 3-way product ----
        mbuf = sb.tile([128, C, R * S], F16, tag="mbuf")
        for c in range(C):
            proj = []
            for m in range(3):
                pool_m = pp0 if m == 0 else pp
                p = pool_m.tile([128, R * S], F32, tag="proj0" if m == 0 else "proj")
                nc.tensor.matmul(
                    p, lhsT=xT_s[:, c, m, bsl], rhs=Wb_s[:, c, m, :],
                    start=True, stop=True,
                )
                proj.append(p)
            # DVE has a single PSUM read port: at most one PSUM operand per
            # tensor_tensor. Evacuate proj0 PSUM->SBUF on ScalarE first.
            p0c = sb.tile([128, R * S], F16, tag="p0c")
            nc.scalar.copy(p0c, proj[0])
            m01 = sb.tile([128, R * S], F32, tag="m01")
            nc.vector.tensor_mul(m01, p0c, proj[1])
            nc.vector.tensor_mul(mbuf[:, c, :], m01, proj[2])
            # rank reduce on GpSimd in groups of 5 chunks (pipelines with
            # the next group's matmuls instead of one big tail barrier)
            if c % 5 == 4:
                g0 = c - 4
                csl = slice(g0, c + 1)
                mbv = mbuf.rearrange("p c (s r) -> p c s r", r=R)
                tr1 = sb.tile([128, 5, S], F16, tag="tr1")
                tr2 = sb.tile([128, 5, S], F16, tag="tr2")
                zbv = zbuf.rearrange("p (c s) -> p c s", s=S)
                nc.gpsimd.tensor_add(tr1, mbv[:, csl, :, 0], mbv[:, csl, :, 1])
                nc.gpsimd.tensor_add(tr2, mbv[:, csl, :, 2], mbv[:, csl, :, 3])
                nc.gpsimd.tensor_add(tr1, tr1, tr2)
                nc.gpsimd.tensor_add(zbv[:, csl, :], tr1, mbv[:, csl, :, 4])
        # ---- phase B: signed sqrt + L2 normalize ----
        # z' = sign(z)*sqrt(|z|); ||z'||^2 = sum_s |z_s|; g = rsqrt(sum|z|)
        abuf = sb.tile([128, C * S], F32, tag="abuf")
        nc.scalar.activation(out=abuf, in_=zbuf, func=AF.Abs)
        sqb = sb.tile([128, C * S], F16, tag="sqb")
        nc.scalar.activation(out=sqb, in_=abuf, func=AF.Sqrt)
        sgb = sb.tile([128, C * S], F16, tag="sgb")
        nc.scalar.activation(out=sgb, in_=zbuf, func=AF.Sign)
        sa = sb.tile([128, C], F32, tag="sa")
        nc.vector.tensor_reduce(
            out=sa, in_=zbv, axis=mybir.AxisListType.X, op=ALU.add,
            apply_absolute_value=True,
        )
        rsa = sb.tile([128, C], F32, tag="rsa")
        nc.vector.reciprocal(rsa, sa)
        g = sb.tile([128, C], F16, tag="g")
        nc.scalar.activation(out=g, in_=rsa, func=AF.Sqrt)  # g = rsqrt(sum|z|)
        zf = sb.tile([128, C * S], F16, tag="zf")
        nc.vector.tensor_mul(zf, sqb, sgb)
        znb = sb.tile([128, C * S], F16, tag="znb")
        gb = bass.AP(
            tensor=g.tensor, offset=g.offset, ap=[g.ap[0], [1, C], [0, S]],
        )
        nc.vector.tensor_mul(
            znb.rearrange("p (c s) -> p c s", s=S),
            zf.rearrange("p (c s) -> p c s", s=S), gb,
        )
        # ---- phase C: heads ----
        # 5 chunks share one PSUM bank for their logits -> one Exp per group
        expb = sb.tile([128, C, O], F32, tag="expb")
        fin_ps = pf.tile([128, O], F32, tag="finps")
        for c5 in range(C // 5):
            P5_ps = ph.tile([128, 5, O], F32, tag="P")
            for j in range(5):
                c = c5 * 5 + j
                zT_ps = pt.tile([S, 128], F16, tag="zT")
                nc.tensor.transpose(zT_ps, znb[:, c * S:(c + 1) * S], ident)
                zTs = sb.tile([81, 128], F16, tag="zTs")
                nc.gpsimd.memset(zTs, 1.0)  # row 80 stays 1.0 (bias row)
                nc.scalar.copy(zTs[:S, :], zT_ps)
                nc.tensor.matmul(
                    P5_ps[:, j, :], lhsT=zTs, rhs=WoT_s[:, c, :],
                    start=True, stop=True,
                )
                # accumulate final logits; bias row only once (c == 0)
                k = 81 if c == 0 else S
                nc.tensor.matmul(
                    fin_ps, lhsT=zTs[:k, :], rhs=WoT_s[:k, c, :],
                    start=(c == 0), stop=(c == C - 1), skip_group_check=True,
                )
            nc.scalar.activation(
                out=expb[:, c5 * 5:(c5 + 1) * 5, :], in_=P5_ps, func=AF.Exp,
            )
        den = sb.tile([128, C], F32, tag="den")
        nc.vector.tensor_reduce(
            out=den, in_=expb, axis=mybir.AxisListType.X, op=ALU.add,
        )
        rden = sb.tile([128, C], F32, tag="rden")
        nc.vector.reciprocal(rden, den)
        outc = sb.tile([128, C * O], F32, tag="outc")
        rdb = bass.AP(
            tensor=rden.tensor, offset=rden.offset,
            ap=[rden.ap[0], [1, C], [0, O]],
        )
        nc.vector.tensor_mul(
            outc.rearrange("p (c o) -> p c o", o=O), expb, rdb,
        )
        nc.sync.dma_start(out=chk[bsl, :], in_=outc)
        fexp = sb.tile([128, O], F32, tag="fexp")
        nc.scalar.activation(out=fexp, in_=fin_ps, func=AF.Exp)
        fden = sb.tile([128, 1], F32, tag="fden")
        nc.vector.tensor_reduce(
            out=fden, in_=fexp, axis=mybir.AxisListType.X, op=ALU.add,
        )
        rfden = sb.tile([128, 1], F32, tag="rfden")
        nc.vector.reciprocal(rfden, fden)
        outf = sb.tile([128, O], F32, tag="outf")
        nc.vector.tensor_scalar_mul(outf, fexp, rfden)
        nc.sync.dma_start(out=fin[bsl, :], in_=outf)


def build():
    global _prog
    if _prog is not None:
        return _prog
    nc = bacc.Bacc("TRN2", target_bir_lowering=False, debug=False)
    from contextlib import ExitStack

    with tile.TileContext(nc) as tc, ExitStack() as ctx:
        _emit(nc, tc, ctx)
    nc.compile()
    _prog = nc
    return nc


def _prep_inputs(x0, x1, x2, W, b, W_out, b_out):
    """Host-side shard + layout prep. Returns per-core input dicts."""
    xs = np.stack([x0, x1, x2]).astype(np.float32)       # [3, B, MM]
    src = xs.reshape(3, NCORES, BL, C, S)
    xTc = np.empty((NCORES, C, 81, 3, BL), np.float16)
    xTc[:, :, :S] = src.transpose(1, 3, 4, 0, 2)         # [core][c][i][m][u]
    xTc[:, :, S:] = 1.0

    W5 = W.reshape(3, C, R, S, S)
    Wb_a = np.empty((C, 81, 3, R * S), np.float16)
    Wb_a[:, :S] = W5.transpose(1, 4, 0, 3, 2).reshape(C, S, 3, R * S)
    Wb_a[:, S] = (
        b.reshape(3, C, R, S).transpose(1, 0, 3, 2).reshape(C, 3, R * S)
    )

    WoT_a = np.empty((81, C, O), np.float16)
    WoT_a[:S] = W_out.reshape(O, C, S).transpose(2, 1, 0)
    WoT_a[S] = b_out[None, :]

    return [
        {"xT": xTc[i], "Wb": Wb_a, "WoT": WoT_a} for i in range(NCORES)
    ]


def run(x0, x1, x2, W, b, W_out, b_out, trace=False):
    nc = build()
    in_maps = _prep_inputs(
        np.asarray(x0), np.asarray(x1), np.asarray(x2), np.asarray(W),
        np.asarray(b), np.asarray(W_out), np.asarray(b_out),
    )
    res = run_bass_kernel_spmd(nc, in_maps, core_ids=list(range(NCORES)), trace=trace)
    final = np.concatenate([r["fin"] for r in res.results], axis=0)
    chunks = np.concatenate(
        [r["chk"].reshape(BL, C, O) for r in res.results], axis=0
    )
    return (final, chunks), res


def kernel(x0, x1, x2, W, b, W_out, b_out):
    (final, chunks), _ = run(x0, x1, x2, W, b, W_out, b_out, trace=False)
    return final, chunks
